# revision 16
# baseline (speedup 1.0000x reference)
"""ExpertsChooseMlp Trainium2 kernel.

Full inputs in, full output out. Sharding: 8 cores = 4 batches x 2 expert-pairs.
Core m handles batch b=m//2 and experts {2g, 2g+1}, g=m%2. Each core computes
pout[T,O] = sum_{e in pair} combine[b,:,e,:] @ mlp_e(dispatch[b,:,e,:]^T @ x[b]);
the host sums the two partials per batch, applies the w2 rank-1 correction and
adds b2.

All four contractions run as fp8-e4m3 DoubleRow matmuls (2 K-planes of 128 per
pass, ~1.97x bf16 throughput at FD=512, LDWEIGHTS fully hidden), fp32 PSUM
accumulation.

Precision design (the output is dominated by the capacity-mean channel
0.5*sum_c y, so any quantization error that is coherent across the capacity
dim passes straight through at ~2.5% while incoherent error averages away
~30x; each coherent channel is therefore computed exactly on the host):
  - dispatch runs on v8 = fp8(dm - 0.5) (zero-mean), and the removed mean
    term 0.5 * w1_true^T colsum(x_true) is folded (fp64, host) into the
    per-partition GELU bias -> kills the x and w1 coherent channels.
  - w2's coherent channel is removed by a host rank-1 correction
    rowsum(cm8) (x) (ghsum @ (w2q/16 - w2))/C, where ghsum = sum_c gelu
    output is measured exactly on-device via activation accum_out (free).
  - w1/w2 are pre-scaled by 16 so their fp8 values avoid subnormals; the
    1/16 unscale is folded into the ScalarE activations (GELU / Copy).
Emulated end-to-end: rel ~ 6.6e-3 (gate 2e-2).

All DMA operands are pre-arranged on the host so device DMAs move >=1KB
contiguous per partition (no on-device rearranges; descriptor-rate limited
DMAs were gating the first matmul). No warmup block: HAM ramps on the first
real phase-A matmuls (~8us).
"""
import sys

sys.path.insert(0, "/opt/trn_rl_repo")

import numpy as np
import ml_dtypes

import concourse.bacc as bacc
import concourse.mybir as mybir
import concourse.tile as tile
from concourse import bass_utils

B, T, D, E, C, HE, O = 4, 2048, 512, 4, 1024, 512, 512
P = 128
nKP = T // (2 * P)  # 8  T pair-chunks (phase A DoubleRow: K=256 per matmul)
nMD = D // P        # 4  D-chunks
nMH = HE // P       # 4  HE-chunks
nKDP = D // (2 * P)   # 2  D pair-chunks (phase B DR)
nKHP = HE // (2 * P)  # 2  HE pair-chunks (phase C DR)
nCC = C // P        # 8  C-chunks
nMT = T // P        # 16
NF = 512            # matmul free dim (one PSUM bank)
WS = 16.0           # host-side w1/w2 scale (keeps fp8 weights out of subnormals)

F32 = mybir.dt.float32
F8 = mybir.dt.float8e4
GELU = mybir.ActivationFunctionType.Gelu
COPY = mybir.ActivationFunctionType.Copy
DR = mybir.MatmulPerfMode.DoubleRow
nCP = nCC // 2      # 4  C pair-chunks (phase D DR)

_NC = None


def _build():
    nc = bacc.Bacc("TRN2", target_bir_lowering=False, debug=False,
                   enable_asserts=False, num_devices=1)
    # host-prearranged layouts: partition dim is explicit so every DMA is
    # contiguous per partition
    xb = nc.dram_tensor("xb", [P, nKP, 2, D], F8, kind="ExternalInput").ap()
    dm = nc.dram_tensor("dm", [2, nKP, P, 2, C], F8, kind="ExternalInput").ap()
    cmt = nc.dram_tensor("cmt", [2, nCP, P, 2, T], F8, kind="ExternalInput").ap()
    w1 = nc.dram_tensor("w1", [P, 2, nKDP, 2, HE], F8, kind="ExternalInput").ap()
    w2 = nc.dram_tensor("w2", [P, 2, nKHP, 2, O], F8, kind="ExternalInput").ap()
    b1 = nc.dram_tensor("b1s", [P, 2 * nMH], F32, kind="ExternalInput").ap()
    pout = nc.dram_tensor("pout", [T, O], F32, kind="ExternalOutput").ap()
    ghs = nc.dram_tensor("ghs", [P, 2, 2, nMH], F32, kind="ExternalOutput").ap()

    with tile.TileContext(nc) as tc:
        with (
            tc.tile_pool(name="const", bufs=1) as const,
            tc.tile_pool(name="dmp", bufs=16) as dmp,
            tc.tile_pool(name="cmp", bufs=8) as cmp_,
            tc.tile_pool(name="inter", bufs=1) as inter,
            tc.tile_pool(name="yp", bufs=2) as yp,
            tc.tile_pool(name="outp", bufs=6) as outp,
            tc.tile_pool(name="gscp", bufs=2) as gscp,
            tc.tile_pool(name="psum", bufs=8, space="PSUM") as psp,
        ):
            # ---- resident constants ----
            # Early DMA streams run ~160 GB/s per ring (packet-dispatch
            # limited) and the scalar ring's packets start ~1us after sync's,
            # so first data can't land before ~10.5us. The two rings run in
            # parallel: dm tiles (both experts) on SYNC, x + weights on
            # scalar with x split 2/2/4 kp so each chunk lands just ahead of
            # its matmuls.
            x_sb = const.tile([P, nKP, 2, D], F8)
            nc.sync.dma_start(x_sb[:, 0, :, :], xb[:, 0, :, :])
            for kp in range(1, nKP):
                nc.scalar.dma_start(x_sb[:, kp, :, :], xb[:, kp, :, :])
            w1_sb = const.tile([P, 2, nKDP, 2, HE], F8)
            nc.scalar.dma_start(w1_sb[:], w1[:])
            w2_sb = const.tile([P, 2, nKHP, 2, O], F8)
            nc.scalar.dma_start(w2_sb[:], w2[:])
            b1_sb = const.tile([P, 2 * nMH], F32)
            nc.scalar.dma_start(b1_sb[:], b1[:])
            acc = const.tile([P, 2, 2, nMH], F32)

            # ---- HAM warmup: ~3us of dummy matmuls on a memset tile during
            # the initial DMA wait so real matmuls start at 2.4GHz ----
            warm = const.tile([P, NF], mybir.dt.bfloat16)
            nc.gpsimd.memset(warm[:], 0.0)
            ps_w = psp.tile([P, NF], F32, tag="ps", name="ps_warm")
            for i in range(7):
                nc.tensor.matmul(ps_w[:], warm[:, 0:P], warm[:],
                                 start=(i == 0), stop=(i == 6))

            y_tiles = []
            for ei in range(2):
                # ---- dispatch-mask pair-tiles for this expert (SYNC ring) ----
                dm_t = []
                for kp in range(nKP):
                    t_ = dmp.tile([P, 2, C], F8, tag="dm")
                    nc.sync.dma_start(t_[:], dm[ei, kp])
                    dm_t.append(t_)

                # ---- phase A: xdT[D, C] = x^T (dm-0.5)  (fp8 DR, K=T) ----
                # kp-outer: all 8 PSUM banks accumulate in parallel, so each
                # dm tile is consumed once (at sustainable DMA rate) and
                # released immediately for the next expert's prefetch. On the
                # last kp the half=0 banks stop first so phase B's operands
                # drain to SBUF before phase A's matmuls finish.
                xdt = inter.tile([P, nMD, C], F8, tag="xdt")
                pss = [psp.tile([P, NF], F32, tag="ps", name=f"psa{i}")
                       for i in range(2 * nMD)]
                for kp in range(nKP):
                    if kp == 0 or kp == nKP - 1:
                        # kp0: the first 4 matmuls only need the first dm half;
                        # last kp: half-0 banks stop first so phase B's
                        # operands drain to SBUF before phase A finishes.
                        order = [(mc, h) for h in range(2) for mc in range(nMD)]
                    else:
                        order = [(mc, h) for mc in range(nMD) for h in range(2)]
                    for mc, h in order:
                        nc.tensor.matmul(pss[2 * mc + h][:],
                                         x_sb[:, kp, :, mc * P:(mc + 1) * P],
                                         dm_t[kp][:, :, h * NF:(h + 1) * NF],
                                         start=(kp == 0), stop=(kp == nKP - 1),
                                         perf_mode=DR)
                for ncc in range(2):
                    for mc in range(nMD):
                        nc.vector.tensor_copy(xdt[:, mc, ncc * NF:(ncc + 1) * NF],
                                              pss[2 * mc + ncc][:])

                # ---- phase B: hT[HE, C] = gelu(w1^T xdT / WS + bias) (DR) ----
                # ncc-outer so phase C's first C-half unblocks after 4 gelus;
                # kp-outer within so the first matmuls only need the first
                # two xdt planes. accum_out captures sum_c gelu exactly for
                # the host-side w2 correction.
                ht = inter.tile([P, nMH, C], F8, tag="ht")
                for ncc in range(2):
                    sl = slice(ncc * NF, (ncc + 1) * NF)
                    bss = [psp.tile([P, NF], F32, tag="ps", name=f"psb{i}")
                           for i in range(nMH)]
                    for kp in range(nKDP):
                        for mh in range(nMH):
                            nc.tensor.matmul(
                                bss[mh][:],
                                w1_sb[:, ei, kp, :, mh * P:(mh + 1) * P],
                                xdt[:, 2 * kp:2 * kp + 2, sl],
                                start=(kp == 0), stop=(kp == nKDP - 1),
                                perf_mode=DR)
                    for mh in range(nMH):
                        bia = b1_sb[:, ei * nMH + mh:ei * nMH + mh + 1]
                        nc.scalar.activation(ht[:, mh, sl], bss[mh][:], GELU,
                                             bias=bia, scale=1.0 / WS)

                # ---- ghsum: sum_c of the fp8 gelu values via DVE pass-through
                # with accum_out (feeds the host-side w2 rank-1 correction) ----
                for ncc in range(2):
                    sl = slice(ncc * NF, (ncc + 1) * NF)
                    for mh in range(nMH):
                        gsc = gscp.tile([P, NF], F8, tag="gsc")
                        nc.vector.tensor_scalar(
                            gsc[:], ht[:, mh, sl], 0.0, None,
                            mybir.AluOpType.add, mybir.AluOpType.add,
                            accum_out=acc[:, ei, ncc, mh:mh + 1])

                # ---- phase C: y[C, O] (fp8 DR; stored fp8, DR plane layout:
                # row c = kp*256 + i*128 + p  ->  y_sb[p, kp, i, :]) ----
                y_sb = yp.tile([P, nCP, 2, O], F8, tag="y")
                for cc in range(nCC):
                    ps = psp.tile([P, NF], F32, tag="ps")
                    for kp in range(nKHP):
                        nc.tensor.matmul(ps[:],
                                         ht[:, 2 * kp:2 * kp + 2, cc * P:(cc + 1) * P],
                                         w2_sb[:, ei, kp, :, :],
                                         start=(kp == 0), stop=(kp == nKHP - 1),
                                         perf_mode=DR)
                    nc.scalar.activation(y_sb[:, cc // 2, cc % 2, :], ps[:],
                                         COPY, scale=1.0 / WS)
                y_tiles.append(y_sb)

            # ---- combine-mask tiles (fp8, [P, plane, T]): SYNC ring behind
            # the dm loads so they can't steal early HBM bandwidth; ghs after
            # them so its dependency on phase B can't stall the cmt stream ----
            cmt_t = {}
            for ei in range(2):
                for kp in range(nCP):
                    t_ = cmp_.tile([P, 2, T], F8, tag="cmt")
                    nc.sync.dma_start(t_[:], cmt[ei, kp])
                    cmt_t[(ei, kp)] = t_
            nc.sync.dma_start(ghs[:], acc[:])

            # ---- phase D: pout[T, O] = sum_e cmT_e^T y_e (fp8 DR) ----
            for mt in range(nMT):
                ps = psp.tile([P, NF], F32, tag="ps")
                idx = 0
                for ei in range(2):
                    for kp in range(nCP):
                        nc.tensor.matmul(ps[:],
                                         cmt_t[(ei, kp)][:, :, mt * P:(mt + 1) * P],
                                         y_tiles[ei][:, kp, :, :],
                                         start=(idx == 0), stop=(idx == 7),
                                         perf_mode=DR)
                        idx += 1
                ot = outp.tile([P, O], F32, tag="out")
                nc.vector.tensor_copy(ot[:], ps[:])
                nc.sync.dma_start(pout[mt * P:(mt + 1) * P, :], ot[:])

    nc.compile()
    return nc


def get_nc():
    global _NC
    if _NC is None:
        _NC = _build()
    return _NC


def make_in_maps(x, dispatch_mask, combine_array, w1, b1, w2):
    f8 = ml_dtypes.float8_e4m3
    in_maps = []
    corrs = []
    for m in range(8):
        b, g = m // 2, m % 2
        es = slice(2 * g, 2 * g + 2)
        # dispatch: v8 = fp8(dm - 0.5), layout [e, kp, p, i, c]
        dm_s = np.transpose(dispatch_mask[b, :, es, :], (1, 0, 2)) - 0.5
        dm_s = np.ascontiguousarray(
            dm_s.reshape(2, nKP, 2, P, C).transpose(0, 1, 3, 2, 4)).astype(f8)
        # combine: fp8(cm), transposed, layout [e, kp, p, i, t]
        cmt_s = np.transpose(combine_array[b, :, es, :], (1, 2, 0))
        cmt_s = np.ascontiguousarray(
            cmt_s.reshape(2, nCP, 2, P, T).transpose(0, 1, 3, 2, 4)).astype(f8)
        # x: fp8, DR plane layout [p, kp, i, d]
        x_s = np.ascontiguousarray(
            x[b].reshape(nKP, 2, P, D).transpose(2, 0, 1, 3)).astype(f8)
        # weights: fp8(16*w), layouts [p, e, kp, i, j]
        w1_s = np.ascontiguousarray(
            (w1[es] * WS).reshape(2, nKDP, 2, P, HE)
            .transpose(3, 0, 1, 2, 4)).astype(f8)
        w2_s = np.ascontiguousarray(
            (w2[es] * WS).reshape(2, nKHP, 2, P, O)
            .transpose(3, 0, 1, 2, 4)).astype(f8)
        # GELU bias: b1 + the exact dispatch-mean term 0.5 * w1^T colsum(x)
        colsum = x[b].astype(np.float64).sum(0)
        bias = (b1[es].astype(np.float64)
                + 0.5 * np.einsum('edh,d->eh', w1[es].astype(np.float64), colsum))
        bias_s = np.ascontiguousarray(
            bias.reshape(2, nMH, P).transpose(2, 0, 1).reshape(P, 2 * nMH)
        ).astype(np.float32)
        # host-side w2 rank-1 correction ingredients
        dw2n = (w2_s.astype(np.float32)
                .transpose(1, 2, 3, 0, 4).reshape(2, HE, O) / WS
                - w2[es])                                    # [2, HE, O]
        rowsum = cmt_s.astype(np.float32).transpose(0, 1, 3, 2, 4) \
            .reshape(2, C, T).sum(1)                         # [2, T] exact from cm8
        corrs.append((dw2n, rowsum))
        in_maps.append({
            "xb": x_s, "dm": dm_s, "cmt": cmt_s,
            "w1": w1_s, "w2": w2_s, "b1s": bias_s,
        })
    return in_maps, corrs


def kernel(x, dispatch_mask, combine_array, w1, b1, w2, b2):
    nc = get_nc()
    x, dispatch_mask, combine_array, w1, b1, w2 = (
        np.asarray(a, dtype=np.float32)
        for a in (x, dispatch_mask, combine_array, w1, b1, w2))
    in_maps, corrs = make_in_maps(x, dispatch_mask, combine_array, w1, b1, w2)
    res = bass_utils.run_bass_kernel_spmd(nc, in_maps, core_ids=list(range(8)))
    b2f = np.asarray(b2, dtype=np.float32)
    out = np.empty((B, T, O), dtype=np.float32)
    for b in range(B):
        out[b] = res.results[2 * b]["pout"] + res.results[2 * b + 1]["pout"] + b2f
        for m in (2 * b, 2 * b + 1):
            dw2n, rowsum = corrs[m]
            g_acc = res.results[m]["ghs"]                    # [P, 2, 2, nMH]
            for ei in range(2):
                # ghsum[h]: h = mh*128 + p, summed over both ncc halves
                ghsum = g_acc[:, ei].sum(1).T.reshape(HE)
                M = (ghsum @ dw2n[ei]) / C                   # [O]
                out[b] -= np.outer(rowsum[ei], M)
    return out


# revision 18
# speedup vs baseline: 1.0293x; 1.0293x over previous
"""ExpertsChooseMlp Trainium2 kernel.

Full inputs in, full output out. Sharding: 8 cores = 4 batches x 2 expert-pairs.
Core m handles batch b=m//2 and experts {2g, 2g+1}, g=m%2. Each core computes
pout[T,O] = sum_{e in pair} combine[b,:,e,:] @ mlp_e(dispatch[b,:,e,:]^T @ x[b]);
the host sums the two partials per batch, applies the w2 rank-1 correction and
adds b2.

All four contractions run as fp8-e4m3 DoubleRow matmuls (2 K-planes of 128 per
pass, ~1.97x bf16 throughput at FD=512, LDWEIGHTS fully hidden), fp32 PSUM
accumulation.

Precision design (the output is dominated by the capacity-mean channel
0.5*sum_c y, so any quantization error that is coherent across the capacity
dim passes straight through at ~2.5% while incoherent error averages away
~30x; each coherent channel is therefore computed exactly on the host):
  - dispatch runs on v8 = fp8(dm - 0.5) (zero-mean), and the removed mean
    term 0.5 * w1_true^T colsum(x_true) is folded (fp64, host) into the
    per-partition GELU bias -> kills the x and w1 coherent channels.
  - w2's coherent channel is removed by a host rank-1 correction
    rowsum(cm8) (x) (ghsum @ (w2q/16 - w2))/C, where ghsum = sum_c gelu
    output is measured exactly on-device via activation accum_out (free).
  - w1/w2 are pre-scaled by 16 so their fp8 values avoid subnormals; the
    1/16 unscale is folded into the ScalarE activations (GELU / Copy).
Emulated end-to-end: rel ~ 6.6e-3 (gate 2e-2).

All DMA operands are pre-arranged on the host so device DMAs move >=1KB
contiguous per partition (no on-device rearranges; descriptor-rate limited
DMAs were gating the first matmul). No warmup block: HAM ramps on the first
real phase-A matmuls (~8us).
"""
import sys

sys.path.insert(0, "/opt/trn_rl_repo")

import numpy as np
import ml_dtypes

import concourse.bacc as bacc
import concourse.mybir as mybir
import concourse.tile as tile
from concourse import bass_utils

B, T, D, E, C, HE, O = 4, 2048, 512, 4, 1024, 512, 512
P = 128
nKP = T // (2 * P)  # 8  T pair-chunks (phase A DoubleRow: K=256 per matmul)
nMD = D // P        # 4  D-chunks
nMH = HE // P       # 4  HE-chunks
nKDP = D // (2 * P)   # 2  D pair-chunks (phase B DR)
nKHP = HE // (2 * P)  # 2  HE pair-chunks (phase C DR)
nCC = C // P        # 8  C-chunks
nMT = T // P        # 16
NF = 512            # matmul free dim (one PSUM bank)
WS = 16.0           # host-side w1/w2 scale (keeps fp8 weights out of subnormals)

F32 = mybir.dt.float32
F8 = mybir.dt.float8e4
GELU = mybir.ActivationFunctionType.Gelu
COPY = mybir.ActivationFunctionType.Copy
DR = mybir.MatmulPerfMode.DoubleRow
nCP = nCC // 2      # 4  C pair-chunks (phase D DR)

_NC = None


def _build():
    nc = bacc.Bacc("TRN2", target_bir_lowering=False, debug=False,
                   enable_asserts=False, num_devices=1)
    # host-prearranged layouts: partition dim is explicit so every DMA is
    # contiguous per partition
    xb = nc.dram_tensor("xb", [P, nKP, 2, D], F8, kind="ExternalInput").ap()
    dm = nc.dram_tensor("dm", [2, nKP, P, 2, C], F8, kind="ExternalInput").ap()
    cmt = nc.dram_tensor("cmt", [2, nCP, P, 2, T], F8, kind="ExternalInput").ap()
    w1 = nc.dram_tensor("w1", [P, 2, nKDP, 2, HE], F8, kind="ExternalInput").ap()
    w2 = nc.dram_tensor("w2", [P, 2, nKHP, 2, O], F8, kind="ExternalInput").ap()
    b1 = nc.dram_tensor("b1s", [P, 2 * nMH], F32, kind="ExternalInput").ap()
    pout = nc.dram_tensor("pout", [T, O], F32, kind="ExternalOutput").ap()
    ghs = nc.dram_tensor("ghs", [P, 2, 2, nMH], F32, kind="ExternalOutput").ap()

    with tile.TileContext(nc) as tc:
        with (
            tc.tile_pool(name="const", bufs=1) as const,
            tc.tile_pool(name="dmp", bufs=16) as dmp,
            tc.tile_pool(name="cmp", bufs=8) as cmp_,
            tc.tile_pool(name="inter", bufs=1) as inter,
            tc.tile_pool(name="yp", bufs=2) as yp,
            tc.tile_pool(name="outp", bufs=6) as outp,
            tc.tile_pool(name="gscp", bufs=2) as gscp,
            tc.tile_pool(name="psum", bufs=8, space="PSUM") as psp,
        ):
            # ---- resident constants ----
            # Early DMA streams run ~160 GB/s per ring (packet-dispatch
            # limited) and the scalar ring's packets start ~1us after sync's,
            # so first data can't land before ~10.5us. The two rings run in
            # parallel: dm tiles (both experts) on SYNC, x + weights on
            # scalar with x split 2/2/4 kp so each chunk lands just ahead of
            # its matmuls.
            x_sb = const.tile([P, nKP, 2, D], F8)
            nc.scalar.dma_start(x_sb[:, 0:2, :, :], xb[:, 0:2, :, :])
            for kp in range(2, nKP):
                nc.scalar.dma_start(x_sb[:, kp, :, :], xb[:, kp, :, :])
            w1_sb = const.tile([P, 2, nKDP, 2, HE], F8)
            nc.scalar.dma_start(w1_sb[:], w1[:])
            w2_sb = const.tile([P, 2, nKHP, 2, O], F8)
            nc.scalar.dma_start(w2_sb[:], w2[:])
            b1_sb = const.tile([P, 2 * nMH], F32)
            nc.scalar.dma_start(b1_sb[:], b1[:])
            acc = const.tile([P, 2, 2, nMH], F32)

            # ---- HAM warmup: ~3us of dummy matmuls on a memset tile during
            # the initial DMA wait so real matmuls start at 2.4GHz ----
            warm = const.tile([P, NF], mybir.dt.bfloat16)
            nc.gpsimd.memset(warm[:], 0.0)
            ps_w = psp.tile([P, NF], F32, tag="ps", name="ps_warm")
            for i in range(8):
                nc.tensor.matmul(ps_w[:], warm[:, 0:P], warm[:],
                                 start=(i == 0), stop=(i == 7))

            y_tiles = []
            for ei in range(2):
                # ---- dispatch-mask pair-tiles for this expert (SYNC ring) ----
                dm_t = []
                for kp in range(nKP):
                    t_ = dmp.tile([P, 2, C], F8, tag="dm")
                    nc.sync.dma_start(t_[:], dm[ei, kp])
                    dm_t.append(t_)

                # ---- phase A: xdT[D, C] = x^T (dm-0.5)  (fp8 DR, K=T) ----
                # kp-outer: all 8 PSUM banks accumulate in parallel, so each
                # dm tile is consumed once (at sustainable DMA rate) and
                # released immediately for the next expert's prefetch. On the
                # last kp the half=0 banks stop first so phase B's operands
                # drain to SBUF before phase A's matmuls finish.
                xdt = inter.tile([P, nMD, C], F8, tag="xdt")
                pss = [psp.tile([P, NF], F32, tag="ps", name=f"psa{i}")
                       for i in range(2 * nMD)]
                for kp in range(nKP):
                    if kp == 0 or kp == nKP - 1:
                        # kp0: the first 4 matmuls only need the first dm half;
                        # last kp: half-0 banks stop first so phase B's
                        # operands drain to SBUF before phase A finishes.
                        order = [(mc, h) for h in range(2) for mc in range(nMD)]
                    else:
                        order = [(mc, h) for mc in range(nMD) for h in range(2)]
                    for mc, h in order:
                        nc.tensor.matmul(pss[2 * mc + h][:],
                                         x_sb[:, kp, :, mc * P:(mc + 1) * P],
                                         dm_t[kp][:, :, h * NF:(h + 1) * NF],
                                         start=(kp == 0), stop=(kp == nKP - 1),
                                         perf_mode=DR)
                for ncc in range(2):
                    for mc in range(nMD):
                        nc.vector.tensor_copy(xdt[:, mc, ncc * NF:(ncc + 1) * NF],
                                              pss[2 * mc + ncc][:])

                # ---- phase B: hT[HE, C] = gelu(w1^T xdT / WS + bias) (DR) ----
                # ncc-outer so phase C's first C-half unblocks after 4 gelus;
                # kp-outer within so the first matmuls only need the first
                # two xdt planes. accum_out captures sum_c gelu exactly for
                # the host-side w2 correction.
                ht = inter.tile([P, nMH, C], F8, tag="ht")
                for ncc in range(2):
                    sl = slice(ncc * NF, (ncc + 1) * NF)
                    bss = [psp.tile([P, NF], F32, tag="ps", name=f"psb{i}")
                           for i in range(nMH)]
                    for kp in range(nKDP):
                        for mh in range(nMH):
                            nc.tensor.matmul(
                                bss[mh][:],
                                w1_sb[:, ei, kp, :, mh * P:(mh + 1) * P],
                                xdt[:, 2 * kp:2 * kp + 2, sl],
                                start=(kp == 0), stop=(kp == nKDP - 1),
                                perf_mode=DR)
                    for mh in range(nMH):
                        bia = b1_sb[:, ei * nMH + mh:ei * nMH + mh + 1]
                        nc.scalar.activation(ht[:, mh, sl], bss[mh][:], GELU,
                                             bias=bia, scale=1.0 / WS)

                # ---- ghsum: sum_c of the fp8 gelu values via DVE pass-through
                # with accum_out (feeds the host-side w2 rank-1 correction) ----
                for ncc in range(2):
                    sl = slice(ncc * NF, (ncc + 1) * NF)
                    for mh in range(nMH):
                        gsc = gscp.tile([P, NF], F8, tag="gsc")
                        nc.vector.tensor_scalar(
                            gsc[:], ht[:, mh, sl], 0.0, None,
                            mybir.AluOpType.add, mybir.AluOpType.add,
                            accum_out=acc[:, ei, ncc, mh:mh + 1])

                # ---- phase C: y[C, O] (fp8 DR; stored fp8, DR plane layout:
                # row c = kp*256 + i*128 + p  ->  y_sb[p, kp, i, :]) ----
                y_sb = yp.tile([P, nCP, 2, O], F8, tag="y")
                for cc in range(nCC):
                    ps = psp.tile([P, NF], F32, tag="ps")
                    for kp in range(nKHP):
                        nc.tensor.matmul(ps[:],
                                         ht[:, 2 * kp:2 * kp + 2, cc * P:(cc + 1) * P],
                                         w2_sb[:, ei, kp, :, :],
                                         start=(kp == 0), stop=(kp == nKHP - 1),
                                         perf_mode=DR)
                    nc.scalar.activation(y_sb[:, cc // 2, cc % 2, :], ps[:],
                                         COPY, scale=1.0 / WS)
                y_tiles.append(y_sb)

            # ---- combine-mask tiles (fp8, [P, plane, T]): SYNC ring behind
            # the dm loads so they can't steal early HBM bandwidth; ghs after
            # them so its dependency on phase B can't stall the cmt stream ----
            cmt_t = {}
            for ei in range(2):
                for kp in range(nCP):
                    t_ = cmp_.tile([P, 2, T], F8, tag="cmt")
                    nc.sync.dma_start(t_[:], cmt[ei, kp])
                    cmt_t[(ei, kp)] = t_
            nc.sync.dma_start(ghs[:], acc[:])

            # ---- phase D: pout[T, O] = sum_e cmT_e^T y_e (fp8 DR) ----
            for mt in range(nMT):
                ps = psp.tile([P, NF], F32, tag="ps")
                idx = 0
                for ei in range(2):
                    for kp in range(nCP):
                        nc.tensor.matmul(ps[:],
                                         cmt_t[(ei, kp)][:, :, mt * P:(mt + 1) * P],
                                         y_tiles[ei][:, kp, :, :],
                                         start=(idx == 0), stop=(idx == 7),
                                         perf_mode=DR)
                        idx += 1
                ot = outp.tile([P, O], F32, tag="out")
                nc.vector.tensor_copy(ot[:], ps[:])
                nc.sync.dma_start(pout[mt * P:(mt + 1) * P, :], ot[:])

    nc.compile()
    return nc


def get_nc():
    global _NC
    if _NC is None:
        _NC = _build()
    return _NC


def make_in_maps(x, dispatch_mask, combine_array, w1, b1, w2):
    f8 = ml_dtypes.float8_e4m3
    in_maps = []
    corrs = []
    for m in range(8):
        b, g = m // 2, m % 2
        es = slice(2 * g, 2 * g + 2)
        # dispatch: v8 = fp8(dm - 0.5), layout [e, kp, p, i, c]
        dm_s = np.transpose(dispatch_mask[b, :, es, :], (1, 0, 2)) - 0.5
        dm_s = np.ascontiguousarray(
            dm_s.reshape(2, nKP, 2, P, C).transpose(0, 1, 3, 2, 4)).astype(f8)
        # combine: fp8(cm), transposed, layout [e, kp, p, i, t]
        cmt_s = np.transpose(combine_array[b, :, es, :], (1, 2, 0))
        cmt_s = np.ascontiguousarray(
            cmt_s.reshape(2, nCP, 2, P, T).transpose(0, 1, 3, 2, 4)).astype(f8)
        # x: fp8, DR plane layout [p, kp, i, d]
        x_s = np.ascontiguousarray(
            x[b].reshape(nKP, 2, P, D).transpose(2, 0, 1, 3)).astype(f8)
        # weights: fp8(16*w), layouts [p, e, kp, i, j]
        w1_s = np.ascontiguousarray(
            (w1[es] * WS).reshape(2, nKDP, 2, P, HE)
            .transpose(3, 0, 1, 2, 4)).astype(f8)
        w2_s = np.ascontiguousarray(
            (w2[es] * WS).reshape(2, nKHP, 2, P, O)
            .transpose(3, 0, 1, 2, 4)).astype(f8)
        # GELU bias: b1 + the exact dispatch-mean term 0.5 * w1^T colsum(x)
        colsum = x[b].astype(np.float64).sum(0)
        bias = (b1[es].astype(np.float64)
                + 0.5 * np.einsum('edh,d->eh', w1[es].astype(np.float64), colsum))
        bias_s = np.ascontiguousarray(
            bias.reshape(2, nMH, P).transpose(2, 0, 1).reshape(P, 2 * nMH)
        ).astype(np.float32)
        # host-side w2 rank-1 correction ingredients
        dw2n = (w2_s.astype(np.float32)
                .transpose(1, 2, 3, 0, 4).reshape(2, HE, O) / WS
                - w2[es])                                    # [2, HE, O]
        rowsum = cmt_s.astype(np.float32).transpose(0, 1, 3, 2, 4) \
            .reshape(2, C, T).sum(1)                         # [2, T] exact from cm8
        corrs.append((dw2n, rowsum))
        in_maps.append({
            "xb": x_s, "dm": dm_s, "cmt": cmt_s,
            "w1": w1_s, "w2": w2_s, "b1s": bias_s,
        })
    return in_maps, corrs


def kernel(x, dispatch_mask, combine_array, w1, b1, w2, b2):
    nc = get_nc()
    x, dispatch_mask, combine_array, w1, b1, w2 = (
        np.asarray(a, dtype=np.float32)
        for a in (x, dispatch_mask, combine_array, w1, b1, w2))
    in_maps, corrs = make_in_maps(x, dispatch_mask, combine_array, w1, b1, w2)
    res = bass_utils.run_bass_kernel_spmd(nc, in_maps, core_ids=list(range(8)))
    b2f = np.asarray(b2, dtype=np.float32)
    out = np.empty((B, T, O), dtype=np.float32)
    for b in range(B):
        out[b] = res.results[2 * b]["pout"] + res.results[2 * b + 1]["pout"] + b2f
        for m in (2 * b, 2 * b + 1):
            dw2n, rowsum = corrs[m]
            g_acc = res.results[m]["ghs"]                    # [P, 2, 2, nMH]
            for ei in range(2):
                # ghsum[h]: h = mh*128 + p, summed over both ncc halves
                ghsum = g_acc[:, ei].sum(1).T.reshape(HE)
                M = (ghsum @ dw2n[ei]) / C                   # [O]
                out[b] -= np.outer(rowsum[ei], M)
    return out


# revision 21
# speedup vs baseline: 1.0310x; 1.0017x over previous
"""ExpertsChooseMlp Trainium2 kernel.

Full inputs in, full output out. Sharding: 8 cores = 4 batches x 2 expert-pairs.
Core m handles batch b=m//2 and experts {2g, 2g+1}, g=m%2. Each core computes
pout[T,O] = sum_{e in pair} combine[b,:,e,:] @ mlp_e(dispatch[b,:,e,:]^T @ x[b]);
the host sums the two partials per batch, applies the w2 rank-1 correction and
adds b2.

All four contractions run as fp8-e4m3 DoubleRow matmuls (2 K-planes of 128 per
pass, ~1.97x bf16 throughput at FD=512, LDWEIGHTS fully hidden), fp32 PSUM
accumulation.

Precision design (the output is dominated by the capacity-mean channel
0.5*sum_c y, so any quantization error that is coherent across the capacity
dim passes straight through at ~2.5% while incoherent error averages away
~30x; each coherent channel is therefore computed exactly on the host):
  - dispatch runs on v8 = fp8(dm - 0.5) (zero-mean), and the removed mean
    term 0.5 * w1_true^T colsum(x_true) is folded (fp64, host) into the
    per-partition GELU bias -> kills the x and w1 coherent channels.
  - w2's coherent channel is removed by a host rank-1 correction
    rowsum(cm8) (x) (ghsum @ (w2q/16 - w2))/C, where ghsum = sum_c gelu
    output is measured exactly on-device via activation accum_out (free).
  - w1/w2 are pre-scaled by 16 so their fp8 values avoid subnormals; the
    1/16 unscale is folded into the ScalarE activations (GELU / Copy).
Emulated end-to-end: rel ~ 6.6e-3 (gate 2e-2).

All DMA operands are pre-arranged on the host so device DMAs move >=1KB
contiguous per partition (no on-device rearranges; descriptor-rate limited
DMAs were gating the first matmul). No warmup block: HAM ramps on the first
real phase-A matmuls (~8us).
"""
import sys

sys.path.insert(0, "/opt/trn_rl_repo")

import numpy as np
import ml_dtypes

import concourse.bacc as bacc
import concourse.mybir as mybir
import concourse.tile as tile
from concourse import bass_utils

B, T, D, E, C, HE, O = 4, 2048, 512, 4, 1024, 512, 512
P = 128
nKP = T // (2 * P)  # 8  T pair-chunks (phase A DoubleRow: K=256 per matmul)
nMD = D // P        # 4  D-chunks
nMH = HE // P       # 4  HE-chunks
nKDP = D // (2 * P)   # 2  D pair-chunks (phase B DR)
nKHP = HE // (2 * P)  # 2  HE pair-chunks (phase C DR)
nCC = C // P        # 8  C-chunks
nMT = T // P        # 16
NF = 512            # matmul free dim (one PSUM bank)
WS = 16.0           # host-side w1/w2 scale (keeps fp8 weights out of subnormals)

F32 = mybir.dt.float32
F8 = mybir.dt.float8e4
GELU = mybir.ActivationFunctionType.Gelu
COPY = mybir.ActivationFunctionType.Copy
DR = mybir.MatmulPerfMode.DoubleRow
nCP = nCC // 2      # 4  C pair-chunks (phase D DR)

_NC = None


def _build():
    nc = bacc.Bacc("TRN2", target_bir_lowering=False, debug=False,
                   enable_asserts=False, num_devices=1)
    # host-prearranged layouts: partition dim is explicit so every DMA is
    # contiguous per partition
    xb = nc.dram_tensor("xb", [P, nKP, 2, D], F8, kind="ExternalInput").ap()
    dm = nc.dram_tensor("dm", [2, nKP, P, 2, C], F8, kind="ExternalInput").ap()
    cmt = nc.dram_tensor("cmt", [2, nCP, P, 2, T], F8, kind="ExternalInput").ap()
    w1 = nc.dram_tensor("w1", [P, 2, nKDP, 2, HE], F8, kind="ExternalInput").ap()
    w2 = nc.dram_tensor("w2", [P, 2, nKHP, 2, O], F8, kind="ExternalInput").ap()
    b1 = nc.dram_tensor("b1s", [P, 2 * nMH], F32, kind="ExternalInput").ap()
    pout = nc.dram_tensor("pout", [T, O], F32, kind="ExternalOutput").ap()
    ghs = nc.dram_tensor("ghs", [P, 2, 2, nMH], F32, kind="ExternalOutput").ap()

    with tile.TileContext(nc) as tc:
        with (
            tc.tile_pool(name="const", bufs=1) as const,
            tc.tile_pool(name="dmp", bufs=16) as dmp,
            tc.tile_pool(name="cmp", bufs=8) as cmp_,
            tc.tile_pool(name="inter", bufs=1) as inter,
            tc.tile_pool(name="yp", bufs=2) as yp,
            tc.tile_pool(name="outp", bufs=6) as outp,
            tc.tile_pool(name="gscp", bufs=2) as gscp,
            tc.tile_pool(name="psum", bufs=8, space="PSUM") as psp,
        ):
            # ---- resident constants ----
            # Early DMA streams run ~160 GB/s per ring (packet-dispatch
            # limited) and the scalar ring's packets start ~1us after sync's,
            # so first data can't land before ~10.5us. The two rings run in
            # parallel: dm tiles (both experts) on SYNC, x + weights on
            # scalar with x split 2/2/4 kp so each chunk lands just ahead of
            # its matmuls.
            x_sb = const.tile([P, nKP, 2, D], F8)
            nc.scalar.dma_start(x_sb[:, 0:2, :, :], xb[:, 0:2, :, :])
            for kp in range(2, nKP):
                nc.scalar.dma_start(x_sb[:, kp, :, :], xb[:, kp, :, :])
            w1_sb = const.tile([P, 2, nKDP, 2, HE], F8)
            nc.scalar.dma_start(w1_sb[:], w1[:])
            w2_sb = const.tile([P, 2, nKHP, 2, O], F8)
            nc.scalar.dma_start(w2_sb[:], w2[:])
            b1_sb = const.tile([P, 2 * nMH], F32)
            nc.scalar.dma_start(b1_sb[:], b1[:])
            acc = const.tile([P, 2, 2, nMH], F32)

            # ---- HAM warmup: ~3us of dummy matmuls on a memset tile during
            # the initial DMA wait so real matmuls start at 2.4GHz ----
            warm = const.tile([P, NF], mybir.dt.bfloat16)
            nc.gpsimd.memset(warm[:], 0.0)
            ps_w = psp.tile([P, NF], F32, tag="ps", name="ps_warm")
            for i in range(9):
                nc.tensor.matmul(ps_w[:], warm[:, 0:P], warm[:],
                                 start=(i == 0), stop=(i == 8))

            y_tiles = []
            for ei in range(2):
                # ---- dispatch-mask pair-tiles for this expert (SYNC ring) ----
                dm_t = []
                for kp in range(nKP):
                    t_ = dmp.tile([P, 2, C], F8, tag="dm")
                    nc.sync.dma_start(t_[:], dm[ei, kp])
                    dm_t.append(t_)

                # ---- phase A: xdT[D, C] = x^T (dm-0.5)  (fp8 DR, K=T) ----
                # kp-outer: all 8 PSUM banks accumulate in parallel, so each
                # dm tile is consumed once (at sustainable DMA rate) and
                # released immediately for the next expert's prefetch. On the
                # last kp the half=0 banks stop first so phase B's operands
                # drain to SBUF before phase A's matmuls finish.
                xdt = inter.tile([P, nMD, C], F8, tag="xdt")
                pss = [psp.tile([P, NF], F32, tag="ps", name=f"psa{i}")
                       for i in range(2 * nMD)]
                for kp in range(nKP):
                    if kp == 0 or kp == nKP - 1:
                        # kp0: the first 4 matmuls only need the first dm half;
                        # last kp: half-0 banks stop first so phase B's
                        # operands drain to SBUF before phase A finishes.
                        order = [(mc, h) for h in range(2) for mc in range(nMD)]
                    else:
                        order = [(mc, h) for mc in range(nMD) for h in range(2)]
                    for mc, h in order:
                        nc.tensor.matmul(pss[2 * mc + h][:],
                                         x_sb[:, kp, :, mc * P:(mc + 1) * P],
                                         dm_t[kp][:, :, h * NF:(h + 1) * NF],
                                         start=(kp == 0), stop=(kp == nKP - 1),
                                         perf_mode=DR)
                for ncc in range(2):
                    for mc in range(nMD):
                        nc.vector.tensor_copy(xdt[:, mc, ncc * NF:(ncc + 1) * NF],
                                              pss[2 * mc + ncc][:])

                # ---- phase B: hT[HE, C] = gelu(w1^T xdT / WS + bias) (DR) ----
                # ncc-outer so phase C's first C-half unblocks after 4 gelus;
                # kp-outer within so the first matmuls only need the first
                # two xdt planes. accum_out captures sum_c gelu exactly for
                # the host-side w2 correction.
                ht = inter.tile([P, nMH, C], F8, tag="ht")
                for ncc in range(2):
                    sl = slice(ncc * NF, (ncc + 1) * NF)
                    bss = [psp.tile([P, NF], F32, tag="ps", name=f"psb{i}")
                           for i in range(nMH)]
                    for kp in range(nKDP):
                        for mh in range(nMH):
                            nc.tensor.matmul(
                                bss[mh][:],
                                w1_sb[:, ei, kp, :, mh * P:(mh + 1) * P],
                                xdt[:, 2 * kp:2 * kp + 2, sl],
                                start=(kp == 0), stop=(kp == nKDP - 1),
                                perf_mode=DR)
                    for mh in range(nMH):
                        bia = b1_sb[:, ei * nMH + mh:ei * nMH + mh + 1]
                        nc.scalar.activation(ht[:, mh, sl], bss[mh][:], GELU,
                                             bias=bia, scale=1.0 / WS)

                # ---- ghsum: sum_c of the fp8 gelu values via DVE pass-through
                # with accum_out (feeds the host-side w2 rank-1 correction) ----
                for ncc in range(2):
                    sl = slice(ncc * NF, (ncc + 1) * NF)
                    for mh in range(nMH):
                        gsc = gscp.tile([P, NF], F8, tag="gsc")
                        nc.vector.tensor_scalar(
                            gsc[:], ht[:, mh, sl], 0.0, None,
                            mybir.AluOpType.add, mybir.AluOpType.add,
                            accum_out=acc[:, ei, ncc, mh:mh + 1])

                # ---- phase C: y[C, O] (fp8 DR; stored fp8, DR plane layout:
                # row c = kp*256 + i*128 + p  ->  y_sb[p, kp, i, :]) ----
                y_sb = yp.tile([P, nCP, 2, O], F8, tag="y")
                for cc in range(nCC):
                    ps = psp.tile([P, NF], F32, tag="ps")
                    for kp in range(nKHP):
                        nc.tensor.matmul(ps[:],
                                         ht[:, 2 * kp:2 * kp + 2, cc * P:(cc + 1) * P],
                                         w2_sb[:, ei, kp, :, :],
                                         start=(kp == 0), stop=(kp == nKHP - 1),
                                         perf_mode=DR)
                    nc.scalar.activation(y_sb[:, cc // 2, cc % 2, :], ps[:],
                                         COPY, scale=1.0 / WS)
                y_tiles.append(y_sb)

            # ---- combine-mask tiles (fp8, [P, plane, T]): SYNC ring behind
            # the dm loads so they can't steal early HBM bandwidth; ghs after
            # them so its dependency on phase B can't stall the cmt stream ----
            cmt_t = {}
            for ei in range(2):
                for kp in range(nCP):
                    t_ = cmp_.tile([P, 2, T], F8, tag="cmt")
                    nc.sync.dma_start(t_[:], cmt[ei, kp])
                    cmt_t[(ei, kp)] = t_
            nc.sync.dma_start(ghs[:], acc[:])

            # ---- phase D: pout[T, O] = sum_e cmT_e^T y_e (fp8 DR) ----
            # The last chunk runs as two half-O groups so the final
            # copy+DMA+HBM-receipt chain after the last matmul is halved.
            for mt in range(nMT):
                halves = 1 if mt < nMT - 1 else 2
                hw_ = O // halves
                for h in range(halves):
                    sl = slice(h * hw_, (h + 1) * hw_)
                    ps = psp.tile([P, hw_], F32, tag="ps", name=f"psd{mt}_{h}")
                    idx = 0
                    for ei in range(2):
                        for kp in range(nCP):
                            nc.tensor.matmul(ps[:],
                                             cmt_t[(ei, kp)][:, :, mt * P:(mt + 1) * P],
                                             y_tiles[ei][:, kp, :, sl],
                                             start=(idx == 0), stop=(idx == 7),
                                             perf_mode=DR)
                            idx += 1
                    ot = outp.tile([P, hw_], F32, tag="out", name=f"ot{mt}_{h}")
                    nc.vector.tensor_copy(ot[:], ps[:])
                    nc.sync.dma_start(pout[mt * P:(mt + 1) * P, sl], ot[:])

    nc.compile()
    return nc


def get_nc():
    global _NC
    if _NC is None:
        _NC = _build()
    return _NC


def make_in_maps(x, dispatch_mask, combine_array, w1, b1, w2):
    f8 = ml_dtypes.float8_e4m3
    in_maps = []
    corrs = []
    for m in range(8):
        b, g = m // 2, m % 2
        es = slice(2 * g, 2 * g + 2)
        # dispatch: v8 = fp8(dm - 0.5), layout [e, kp, p, i, c]
        dm_s = np.transpose(dispatch_mask[b, :, es, :], (1, 0, 2)) - 0.5
        dm_s = np.ascontiguousarray(
            dm_s.reshape(2, nKP, 2, P, C).transpose(0, 1, 3, 2, 4)).astype(f8)
        # combine: fp8(cm), transposed, layout [e, kp, p, i, t]
        cmt_s = np.transpose(combine_array[b, :, es, :], (1, 2, 0))
        cmt_s = np.ascontiguousarray(
            cmt_s.reshape(2, nCP, 2, P, T).transpose(0, 1, 3, 2, 4)).astype(f8)
        # x: fp8, DR plane layout [p, kp, i, d]
        x_s = np.ascontiguousarray(
            x[b].reshape(nKP, 2, P, D).transpose(2, 0, 1, 3)).astype(f8)
        # weights: fp8(16*w), layouts [p, e, kp, i, j]
        w1_s = np.ascontiguousarray(
            (w1[es] * WS).reshape(2, nKDP, 2, P, HE)
            .transpose(3, 0, 1, 2, 4)).astype(f8)
        w2_s = np.ascontiguousarray(
            (w2[es] * WS).reshape(2, nKHP, 2, P, O)
            .transpose(3, 0, 1, 2, 4)).astype(f8)
        # GELU bias: b1 + the exact dispatch-mean term 0.5 * w1^T colsum(x)
        colsum = x[b].astype(np.float64).sum(0)
        bias = (b1[es].astype(np.float64)
                + 0.5 * np.einsum('edh,d->eh', w1[es].astype(np.float64), colsum))
        bias_s = np.ascontiguousarray(
            bias.reshape(2, nMH, P).transpose(2, 0, 1).reshape(P, 2 * nMH)
        ).astype(np.float32)
        # host-side w2 rank-1 correction ingredients
        dw2n = (w2_s.astype(np.float32)
                .transpose(1, 2, 3, 0, 4).reshape(2, HE, O) / WS
                - w2[es])                                    # [2, HE, O]
        rowsum = cmt_s.astype(np.float32).transpose(0, 1, 3, 2, 4) \
            .reshape(2, C, T).sum(1)                         # [2, T] exact from cm8
        corrs.append((dw2n, rowsum))
        in_maps.append({
            "xb": x_s, "dm": dm_s, "cmt": cmt_s,
            "w1": w1_s, "w2": w2_s, "b1s": bias_s,
        })
    return in_maps, corrs


def kernel(x, dispatch_mask, combine_array, w1, b1, w2, b2):
    nc = get_nc()
    x, dispatch_mask, combine_array, w1, b1, w2 = (
        np.asarray(a, dtype=np.float32)
        for a in (x, dispatch_mask, combine_array, w1, b1, w2))
    in_maps, corrs = make_in_maps(x, dispatch_mask, combine_array, w1, b1, w2)
    res = bass_utils.run_bass_kernel_spmd(nc, in_maps, core_ids=list(range(8)))
    b2f = np.asarray(b2, dtype=np.float32)
    out = np.empty((B, T, O), dtype=np.float32)
    for b in range(B):
        out[b] = res.results[2 * b]["pout"] + res.results[2 * b + 1]["pout"] + b2f
        for m in (2 * b, 2 * b + 1):
            dw2n, rowsum = corrs[m]
            g_acc = res.results[m]["ghs"]                    # [P, 2, 2, nMH]
            for ei in range(2):
                # ghsum[h]: h = mh*128 + p, summed over both ncc halves
                ghsum = g_acc[:, ei].sum(1).T.reshape(HE)
                M = (ghsum @ dw2n[ei]) / C                   # [O]
                out[b] -= np.outer(rowsum[ei], M)
    return out
